# revision 9
# baseline (speedup 1.0000x reference)
"""BiMamba block Trainium2 kernel (8 NeuronCores, SPMD).  v2

Problem: x (2, 1024, 1024) -> bidirectional Mamba (fwd + bwd over flipped
sequence, independent weights) summed, then LayerNorm over d_model.

Sharding: core = (batch b, d_inner quarter q).  Each core processes BOTH
directions for its 512 of 2048 inner channels over the full sequence
(L = 1024), so the selective scan never crosses cores.

v2 structure (vs v1): per-direction AllReduce so fwd scans start while bwd
in_proj still runs on PE (keeps PE dense -> HAM stays warm); dBu multiply on
GPSIMD and 1/4 of the h*C multiplies on GPSIMD to relieve DVE (the
tensor_tensor_scan runs at ~2.2 cyc/el and dominates DVE); out_proj(fwd)
runs under the bwd scans into an SBUF accumulator; ReduceScatter payload is
fp16; dt_proj in bf16.
"""

import sys

sys.path.insert(0, "/opt/trn_rl_repo")

import numpy as np
import ml_dtypes

import concourse.bass as bass
import concourse.tile as tile
import concourse.mybir as mybir
from concourse import bacc
from concourse.bass import ts
from concourse.bass_utils import run_bass_kernel_spmd
from concourse.masks import make_identity

AF = mybir.ActivationFunctionType
ALU = mybir.AluOpType
bf16 = mybir.dt.bfloat16
f16 = mybir.dt.float16
f32 = mybir.dt.float32

B_, L, DM = 2, 1024, 1024
DI, NST, DCONV, DTR = 2048, 16, 4, 64
DQ = 512          # d_inner channels per core
J = DQ // 128     # 4 partition tiles per core
NK = DM // 128    # 8 contraction tiles for d_model
EPS = 1e-5
DIRS = ("f", "b")

_cached = {}


def build(no_collectives=False):
    nc = bacc.Bacc("TRN2", target_bir_lowering=False, debug=False, num_devices=8)

    inp = {}
    for d in DIRS:
        inp[f"xT_{d}"] = nc.dram_tensor(f"xT_{d}", [DM, L], bf16, kind="ExternalInput")
        inp[f"wxiT_{d}"] = nc.dram_tensor(f"wxiT_{d}", [DM, DQ], bf16, kind="ExternalInput")
        inp[f"wzT_{d}"] = nc.dram_tensor(f"wzT_{d}", [DM, DQ], bf16, kind="ExternalInput")
        inp[f"convw_{d}"] = nc.dram_tensor(f"convw_{d}", [DQ, DCONV], f32, kind="ExternalInput")
        inp[f"convb_{d}"] = nc.dram_tensor(f"convb_{d}", [DQ, 1], f32, kind="ExternalInput")
        inp[f"wxpT_{d}"] = nc.dram_tensor(f"wxpT_{d}", [DQ, 96], bf16, kind="ExternalInput")
        inp[f"wdtT_{d}"] = nc.dram_tensor(f"wdtT_{d}", [DTR, DQ], bf16, kind="ExternalInput")
        inp[f"dtb_{d}"] = nc.dram_tensor(f"dtb_{d}", [DQ, 1], f32, kind="ExternalInput")
        inp[f"aneg_{d}"] = nc.dram_tensor(f"aneg_{d}", [DQ, NST], f32, kind="ExternalInput")
        inp[f"dv_{d}"] = nc.dram_tensor(f"dv_{d}", [DQ, 1], f32, kind="ExternalInput")
        inp[f"woutT_{d}"] = nc.dram_tensor(f"woutT_{d}", [DQ, DM], bf16, kind="ExternalInput")
    inp["gamma"] = nc.dram_tensor("gamma", [1, DM], f32, kind="ExternalInput")
    inp["beta"] = nc.dram_tensor("beta", [1, DM], f32, kind="ExternalInput")
    out_q = nc.dram_tensor("out_q", [L // 4, DM], f32, kind="ExternalOutput")

    QUADS = [[0, 1, 2, 3], [4, 5, 6, 7]]

    with tile.TileContext(nc) as tc:
        glob = tc.alloc_tile_pool(name="glob", bufs=1)
        dram = tc.alloc_tile_pool(name="dram", bufs=1, space="DRAM")
        pmm = tc.alloc_tile_pool(name="pmm", bufs=3, space="PSUM")
        py = tc.alloc_tile_pool(name="py", bufs=4, space="PSUM")

        # ---- persistent constants / activations ----
        ident = glob.tile([128, 128], bf16)
        make_identity(nc, ident)
        convb_sb, dtb_sb, dv_sb, aneg_sb, diagD = {}, {}, {}, {}, {}
        for d in DIRS:
            convb_sb[d] = glob.tile([128, J], f32, tag=f"convb{d}", name=f"convb{d}")
            nc.sync.dma_start(out=convb_sb[d],
                              in_=inp[f"convb_{d}"].rearrange("(j p) o -> p (j o)", p=128))
            dtb_sb[d] = glob.tile([128, J], f32, tag=f"dtb{d}", name=f"dtb{d}")
            nc.sync.dma_start(out=dtb_sb[d],
                              in_=inp[f"dtb_{d}"].rearrange("(j p) o -> p (j o)", p=128))
            dv_sb[d] = glob.tile([128, J], f32, tag=f"dv{d}", name=f"dv{d}")
            nc.sync.dma_start(out=dv_sb[d],
                              in_=inp[f"dv_{d}"].rearrange("(j p) o -> p (j o)", p=128))
            aneg_sb[d] = glob.tile([128, J, NST], f32, tag=f"aneg{d}", name=f"aneg{d}")
            nc.sync.dma_start(out=aneg_sb[d],
                              in_=inp[f"aneg_{d}"].rearrange("(j p) n -> p j n", p=128))
            diagD[d] = glob.tile([128, J, 128], bf16, tag=f"diagD{d}", name=f"diagD{d}")
            for j in range(J):
                nc.vector.tensor_scalar_mul(out=diagD[d][:, j, :], in0=ident,
                                            scalar1=dv_sb[d][:, j:j + 1])

        xc_sb = {d: glob.tile([128, J, L], bf16, tag=f"xc{d}", name=f"xc{d}") for d in DIRS}
        zs_sb = {d: glob.tile([128, J, L], bf16, tag=f"zs{d}", name=f"zs{d}") for d in DIRS}
        yg_sb = {d: glob.tile([128, J, L], bf16, tag=f"yg{d}", name=f"yg{d}") for d in DIRS}
        dbl_sb = {d: glob.tile([96, L], f32, tag=f"dbl{d}", name=f"dbl{d}") for d in DIRS}
        oF = glob.tile([128, 8, DM], f32)      # fwd out_proj accumulator [t, dm]

        dbl_in = {d: dram.tile([96, L], f32, tag=f"dbli{d}", name=f"dbli{d}") for d in DIRS}
        dbl_out = {d: dram.tile([96, L], f32, tag=f"dblo{d}", name=f"dblo{d}") for d in DIRS}
        bc_dram = {d: dram.tile([32, L], bf16, tag=f"bcd{d}", name=f"bcd{d}") for d in DIRS}
        op_in = dram.tile([L, DM], f16)
        op_out = dram.tile([L // 4, DM], f16)

        # ================= phase A (per dir): in_proj, conv, x_proj =========
        with tc.tile_pool(name="pA", bufs=1) as pA:
            xT_sb = {}
            for d in DIRS:
                xT_sb[d] = pA.tile([128, NK, L], bf16, tag=f"xT{d}", name=f"xT{d}")
                nc.sync.dma_start(out=xT_sb[d],
                                  in_=inp[f"xT_{d}"].rearrange("(k p) t -> p k t", p=128))

            for d in DIRS:
                convw = pA.tile([128, J, DCONV], f32, tag="convw", name="convw")
                nc.sync.dma_start(out=convw,
                                  in_=inp[f"convw_{d}"].rearrange("(j p) k -> p j k", p=128))
                diagW = pA.tile([128, J, DCONV, 128], bf16, tag="diagW", name="diagW")
                for j in range(J):
                    for k in range(DCONV):
                        nc.vector.tensor_scalar_mul(out=diagW[:, j, k, :], in0=ident,
                                                    scalar1=convw[:, j, k:k + 1])
                wxi = pA.tile([128, NK, DQ], bf16, tag="wxi", name="wxi")
                nc.sync.dma_start(out=wxi,
                                  in_=inp[f"wxiT_{d}"].rearrange("(k p) c -> p k c", p=128))
                wxp = pA.tile([128, J, 96], bf16, tag="wxp", name="wxp")
                nc.sync.dma_start(out=wxp,
                                  in_=inp[f"wxpT_{d}"].rearrange("(j p) r -> p j r", p=128))

                xi_sb = {}
                for j in range(J):
                    xi_sb[j] = pA.tile([128, L + 3], bf16, tag=f"xi{j}", name=f"xi{j}")
                    nc.vector.memset(xi_sb[j][:, 0:3], 0.0)
                    for tb in range(2):
                        pxi = pmm.tile([128, 512], f32, tag="mm", name="mm")
                        for k in range(NK):
                            nc.tensor.matmul(out=pxi[:, :],
                                             lhsT=wxi[:, k, ts(j, 128)],
                                             rhs=xT_sb[d][:, k, ts(tb, 512)],
                                             start=(k == 0), stop=(k == NK - 1))
                        nc.scalar.copy(out=xi_sb[j][:, 3 + tb * 512: 3 + (tb + 1) * 512],
                                       in_=pxi[:, :])
                    # causal depthwise conv as diag-matmuls, silu(+bias)
                    for tb in range(2):
                        pxc = pmm.tile([128, 512], f32, tag="mm", name="mm")
                        for k in range(DCONV):
                            nc.tensor.matmul(out=pxc[:, :],
                                             lhsT=diagW[:, j, k, :],
                                             rhs=xi_sb[j][:, tb * 512 + k: tb * 512 + k + 512],
                                             start=(k == 0), stop=(k == DCONV - 1))
                        nc.scalar.activation(out=xc_sb[d][:, j, ts(tb, 512)], in_=pxc[:, :],
                                             func=AF.Silu, bias=convb_sb[d][:, j:j + 1])
                # x_proj partial + immediate per-dir AllReduce
                dblp = pA.tile([96, L], f32, tag="dblp", name="dblp")
                for tb in range(2):
                    pdb = pmm.tile([96, 512], f32, tag="mm", name="mm")
                    for j in range(J):
                        nc.tensor.matmul(out=pdb[:, :],
                                         lhsT=wxp[:, j, :],
                                         rhs=xc_sb[d][:, j, ts(tb, 512)],
                                         start=(j == 0), stop=(j == J - 1))
                    nc.scalar.copy(out=dblp[:, ts(tb, 512)], in_=pdb[:, :])
                nc.sync.dma_start(out=dbl_in[d][:, :], in_=dblp)
                if no_collectives:
                    nc.sync.dma_start(out=dbl_out[d][:, :], in_=dbl_in[d][:, :])
                else:
                    nc.gpsimd.collective_compute(
                        "AllReduce", ALU.add, replica_groups=QUADS,
                        ins=[dbl_in[d].opt()], outs=[dbl_out[d].opt()])
                nc.sync.dma_start(out=dbl_sb[d], in_=dbl_out[d][:, :])
                bcb = pA.tile([32, L], bf16, tag="bcb", name="bcb")
                nc.scalar.copy(out=bcb, in_=dbl_sb[d][64:96, :])
                nc.sync.dma_start(out=bc_dram[d][:, :], in_=bcb)

                # z projection + silu (after x_proj so the AllReduce fires early)
                wz = pA.tile([128, NK, DQ], bf16, tag="wz", name="wz")
                nc.sync.dma_start(out=wz,
                                  in_=inp[f"wzT_{d}"].rearrange("(k p) c -> p k c", p=128))
                for j in range(J):
                    for tb in range(2):
                        pz = pmm.tile([128, 512], f32, tag="mm", name="mm")
                        for k in range(NK):
                            nc.tensor.matmul(out=pz[:, :],
                                             lhsT=wz[:, k, ts(j, 128)],
                                             rhs=xT_sb[d][:, k, ts(tb, 512)],
                                             start=(k == 0), stop=(k == NK - 1))
                        nc.scalar.activation(out=zs_sb[d][:, j, ts(tb, 512)], in_=pz[:, :],
                                             func=AF.Silu)

        # ================= phase B (per dir): dt_proj, scans, gate, out_proj
        wout = {}
        for d in DIRS:
            wout[d] = glob.tile([128, J, DM], bf16, tag=f"wout{d}", name=f"wout{d}")
            nc.sync.dma_start(out=wout[d],
                              in_=inp[f"woutT_{d}"].rearrange("(j p) m -> p j m", p=128))

        with tc.tile_pool(name="pS", bufs=1) as pS:
            for d in DIRS:
                wdt = pS.tile([DTR, DQ], bf16, tag="wdt", name="wdt")
                nc.sync.dma_start(out=wdt, in_=inp[f"wdtT_{d}"][:, :])
                dtl = pS.tile([DTR, L], bf16, tag="dtl", name="dtl")
                nc.scalar.copy(out=dtl, in_=dbl_sb[d][0:DTR, :])

                dt_sb, dtxc_sb = {}, {}
                for jp in range(2):          # j-pair rounds (PSUM pressure)
                    js = (2 * jp, 2 * jp + 1)
                    for j in js:
                        dt_sb[j] = pS.tile([128, L], f32, tag=f"dt{j}", name=f"dt{j}")
                        for tb in range(2):
                            pdt = pmm.tile([128, 512], f32, tag="mm", name="mm")
                            nc.tensor.matmul(out=pdt[:, :],
                                             lhsT=wdt[:, ts(j, 128)],
                                             rhs=dtl[:, ts(tb, 512)],
                                             start=True, stop=True)
                            # softplus(x) = ln(exp(x) + 1)
                            dte = pS.tile([128, 512], f32, tag="dte", name="dte",
                                          bufs=2)
                            nc.scalar.activation(out=dte, in_=pdt[:, :], func=AF.Exp,
                                                 bias=dtb_sb[d][:, j:j + 1])
                            nc.scalar.activation(out=dt_sb[j][:, ts(tb, 512)],
                                                 in_=dte, func=AF.Ln, bias=1.0)
                        dtxc_sb[j] = pS.tile([128, L], bf16, tag=f"dtxc{j}",
                                             name=f"dtxc{j}")
                        nc.vector.tensor_tensor(out=dtxc_sb[j],
                                                in0=dt_sb[j],
                                                in1=xc_sb[d][:, j, :], op=ALU.mult)

                    pyt = {}
                    for j in js:
                        for tb in range(2):
                            pyt[(j, tb)] = py.tile([128, 512], f32, tag="y", name="y")

                    for n in range(NST):
                        brep = pS.tile([128, L], bf16, tag="brep", name="brep", bufs=3)
                        nc.sync.dma_start(
                            out=brep,
                            in_=bass.AP(tensor=bc_dram[d].tensor,
                                        offset=bc_dram[d].opt().offset + n * L,
                                        ap=[[0, 128], [1, L]]))
                        crep = pS.tile([128, L], bf16, tag="crep", name="crep", bufs=3)
                        nc.sync.dma_start(
                            out=crep,
                            in_=bass.AP(tensor=bc_dram[d].tensor,
                                        offset=bc_dram[d].opt().offset + (NST + n) * L,
                                        ap=[[0, 128], [1, L]]))
                        for j in js:
                            dA = pS.tile([128, L], f32, tag="dA", name="dA", bufs=3)
                            nc.scalar.activation(out=dA, in_=dt_sb[j], func=AF.Exp,
                                                 scale=aneg_sb[d][:, j, n:n + 1])
                            dBu = pS.tile([128, L], bf16, tag="dBu", name="dBu", bufs=3)
                            nc.gpsimd.tensor_tensor(out=dBu, in0=dtxc_sb[j],
                                                    in1=brep, op=ALU.mult)
                            h = pS.tile([128, L], bf16, tag="h", name="h", bufs=3)
                            nc.vector.tensor_tensor_scan(out=h, data0=dA, data1=dBu,
                                                         initial=0.0, op0=ALU.mult,
                                                         op1=ALU.add)
                            hC = pS.tile([128, L], bf16, tag="hC", name="hC", bufs=3)
                            eng = nc.gpsimd if (n % 4 == 3) else nc.vector
                            eng.tensor_tensor(out=hC, in0=h, in1=crep, op=ALU.mult)
                            for tb in range(2):
                                nc.tensor.matmul(out=pyt[(j, tb)][:, :], lhsT=ident,
                                                 rhs=hC[:, ts(tb, 512)],
                                                 start=(n == 0), stop=False)
                    # skip term D*xc, gate with silu(z)
                    for j in js:
                        for tb in range(2):
                            nc.tensor.matmul(out=pyt[(j, tb)][:, :],
                                             lhsT=diagD[d][:, j, :],
                                             rhs=xc_sb[d][:, j, ts(tb, 512)],
                                             start=False, stop=True)
                            nc.vector.tensor_tensor(out=yg_sb[d][:, j, ts(tb, 512)],
                                                    in0=pyt[(j, tb)][:, :],
                                                    in1=zs_sb[d][:, j, ts(tb, 512)],
                                                    op=ALU.mult)

                # ---- out_proj for this direction ----
                if d == "f":
                    # fwd: straight into the SBUF accumulator, overlaps bwd scans
                    for ti in range(8):
                        for mh in range(2):
                            po = pmm.tile([128, 512], f32, tag="mm", name="mm")
                            for j in range(J):
                                nc.tensor.matmul(out=po[:, :],
                                                 lhsT=yg_sb["f"][:, j, ts(ti, 128)],
                                                 rhs=wout["f"][:, j, ts(mh, 512)],
                                                 start=(j == 0), stop=(j == J - 1))
                            nc.scalar.copy(out=oF[:, ti, ts(mh, 512)], in_=po[:, :])
                else:
                    # bwd: flip to global t, matmul, add fwd accumulator, fp16 out
                    ygflip = pS.tile([128, J, L], bf16, tag="ygflip", name="ygflip")
                    for j in range(J):
                        nc.vector.tensor_copy(out=ygflip[:, j, :],
                                              in_=yg_sb["b"][:, j, ::-1])
                    for ti in range(8):
                        for mh in range(2):
                            po = pmm.tile([128, 512], f32, tag="mm", name="mm")
                            for j in range(J):
                                nc.tensor.matmul(out=po[:, :],
                                                 lhsT=ygflip[:, j, ts(ti, 128)],
                                                 rhs=wout["b"][:, j, ts(mh, 512)],
                                                 start=(j == 0), stop=(j == J - 1))
                            osb = pS.tile([128, 512], f16, tag="osb", name="osb",
                                          bufs=3)
                            nc.vector.tensor_tensor(out=osb, in0=po[:, :],
                                                    in1=oF[:, ti, ts(mh, 512)],
                                                    op=ALU.add)
                            nc.sync.dma_start(out=op_in[ts(ti, 128), ts(mh, 512)],
                                              in_=osb)

        # ================= ReduceScatter + LayerNorm ========================
        with tc.tile_pool(name="pO", bufs=1) as pO:
            if no_collectives:
                nc.sync.dma_start(out=op_out[:, :], in_=op_in[0:L // 4, :])
            else:
                nc.gpsimd.collective_compute("ReduceScatter", ALU.add,
                                             replica_groups=QUADS,
                                             ins=[op_in.opt()], outs=[op_out.opt()])

            epst = pO.tile([128, 1], f32, tag="epst", name="epst")
            nc.vector.memset(epst, EPS)
            grep = pO.tile([128, DM], f32, tag="grep", name="grep")
            nc.sync.dma_start(out=grep, in_=bass.AP(tensor=inp["gamma"][:, :].tensor,
                                                    offset=0, ap=[[0, 128], [1, DM]]))
            brep2 = pO.tile([128, DM], f32, tag="brep2", name="brep2")
            nc.sync.dma_start(out=brep2, in_=bass.AP(tensor=inp["beta"][:, :].tensor,
                                                     offset=0, ap=[[0, 128], [1, DM]]))
            for half in range(2):
                xln = pO.tile([128, DM], f32, tag="xln", name="xln", bufs=2)
                nc.gpsimd.dma_start(out=xln, in_=op_out[ts(half, 128), :])
                stats = pO.tile([128, 2, 6], f32, tag="stats", name="stats", bufs=2)
                for sg in range(2):
                    nc.vector.bn_stats(out=stats[:, sg, :], in_=xln[:, ts(sg, 512)])
                mv = pO.tile([128, 2], f32, tag="mv", name="mv", bufs=2)
                nc.vector.bn_aggr(out=mv, in_=stats)
                rstd = pO.tile([128, 1], f32, tag="rstd", name="rstd", bufs=2)
                nc.scalar.activation(out=rstd, in_=mv[:, 1:2], func=AF.Sqrt,
                                     bias=epst[:, 0:1])
                nc.vector.reciprocal(out=rstd, in_=rstd)
                xn = pO.tile([128, DM], f32, tag="xn", name="xn", bufs=2)
                nc.vector.tensor_scalar(out=xn, in0=xln, scalar1=mv[:, 0:1],
                                        scalar2=rstd, op0=ALU.subtract, op1=ALU.mult)
                nc.vector.tensor_tensor(out=xn, in0=xn, in1=grep, op=ALU.mult)
                nc.vector.tensor_tensor(out=xn, in0=xn, in1=brep2, op=ALU.add)
                nc.sync.dma_start(out=out_q[ts(half, 128), :], in_=xn)

        py.release()
        pmm.release()
        dram.release()
        glob.release()

    nc.compile()
    return nc


def _prep_in_maps(inputs):
    x = np.asarray(inputs["x"], np.float32)
    in_maps = []
    for b in range(B_):
        xTf = np.ascontiguousarray(x[b].T).astype(ml_dtypes.bfloat16)
        xTb = np.ascontiguousarray(x[b][::-1, :].T).astype(ml_dtypes.bfloat16)
        for q in range(4):
            sl = slice(q * DQ, (q + 1) * DQ)
            m = {"xT_f": xTf, "xT_b": xTb,
                 "gamma": np.asarray(inputs["ln_gamma"], np.float32).reshape(1, DM),
                 "beta": np.asarray(inputs["ln_beta"], np.float32).reshape(1, DM)}
            for d, pref in (("f", "fwd_"), ("b", "bwd_")):
                W = {k[len(pref):]: np.asarray(v, np.float32)
                     for k, v in inputs.items() if k.startswith(pref)}
                m[f"wxiT_{d}"] = np.ascontiguousarray(W["in_proj_w"][sl, :].T).astype(ml_dtypes.bfloat16)
                m[f"wzT_{d}"] = np.ascontiguousarray(W["in_proj_w"][DI + q * DQ: DI + (q + 1) * DQ, :].T).astype(ml_dtypes.bfloat16)
                m[f"convw_{d}"] = np.ascontiguousarray(W["conv_w"][sl])
                m[f"convb_{d}"] = np.ascontiguousarray(W["conv_b"][sl]).reshape(DQ, 1)
                m[f"wxpT_{d}"] = np.ascontiguousarray(W["x_proj_w"][:, sl].T).astype(ml_dtypes.bfloat16)
                m[f"wdtT_{d}"] = np.ascontiguousarray(W["dt_proj_w"][sl, :].T).astype(ml_dtypes.bfloat16)
                m[f"dtb_{d}"] = np.ascontiguousarray(W["dt_proj_b"][sl]).reshape(DQ, 1)
                m[f"aneg_{d}"] = np.ascontiguousarray(-np.exp(W["A_log"][sl]))
                m[f"dv_{d}"] = np.ascontiguousarray(W["D"][sl]).reshape(DQ, 1)
                m[f"woutT_{d}"] = np.ascontiguousarray(W["out_proj_w"][:, sl].T).astype(ml_dtypes.bfloat16)
            in_maps.append(m)
    return in_maps


def kernel(**inputs):
    if "nc" not in _cached:
        _cached["nc"] = build()
    nc = _cached["nc"]
    in_maps = _prep_in_maps(inputs)
    res = run_bass_kernel_spmd(nc, in_maps, core_ids=list(range(8)),
                               **_cached.get("run_kwargs", {}))
    _cached["last_result"] = res
    out = np.empty((B_, L, DM), np.float32)
    for b in range(B_):
        for q in range(4):
            out[b, q * 256:(q + 1) * 256, :] = res.results[4 * b + q]["out_q"]
    return out


# revision 12
# speedup vs baseline: 1.2744x; 1.2744x over previous
"""BiMamba block Trainium2 kernel (8 NeuronCores, SPMD).  v2

Problem: x (2, 1024, 1024) -> bidirectional Mamba (fwd + bwd over flipped
sequence, independent weights) summed, then LayerNorm over d_model.

Sharding: core = (batch b, d_inner quarter q).  Each core processes BOTH
directions for its 512 of 2048 inner channels over the full sequence
(L = 1024), so the selective scan never crosses cores.

v2 structure (vs v1): per-direction AllReduce so fwd scans start while bwd
in_proj still runs on PE (keeps PE dense -> HAM stays warm); dBu multiply on
GPSIMD and 1/4 of the h*C multiplies on GPSIMD to relieve DVE (the
tensor_tensor_scan runs at ~2.2 cyc/el and dominates DVE); out_proj(fwd)
runs under the bwd scans into an SBUF accumulator; ReduceScatter payload is
fp16; dt_proj in bf16.
"""

import sys

sys.path.insert(0, "/opt/trn_rl_repo")

import numpy as np
import ml_dtypes

import concourse.bass as bass
import concourse.tile as tile
import concourse.mybir as mybir
from concourse import bacc
from concourse.bass import ts
from concourse.bass_utils import run_bass_kernel_spmd
from concourse.masks import make_identity

AF = mybir.ActivationFunctionType
ALU = mybir.AluOpType
bf16 = mybir.dt.bfloat16
f16 = mybir.dt.float16
f32 = mybir.dt.float32

B_, L, DM = 2, 1024, 1024
DI, NST, DCONV, DTR = 2048, 16, 4, 64
DQ = 512          # d_inner channels per core
J = DQ // 128     # 4 partition tiles per core
NK = DM // 128    # 8 contraction tiles for d_model
EPS = 1e-5
DIRS = ("f", "b")

_cached = {}


def build(no_collectives=False):
    nc = bacc.Bacc("TRN2", target_bir_lowering=False, debug=False, num_devices=8)

    inp = {}
    for d in DIRS:
        inp[f"xT_{d}"] = nc.dram_tensor(f"xT_{d}", [DM, L], bf16, kind="ExternalInput")
        inp[f"wxiT_{d}"] = nc.dram_tensor(f"wxiT_{d}", [DM, DQ], bf16, kind="ExternalInput")
        inp[f"wzT_{d}"] = nc.dram_tensor(f"wzT_{d}", [DM, DQ], bf16, kind="ExternalInput")
        inp[f"convw_{d}"] = nc.dram_tensor(f"convw_{d}", [DQ, DCONV], f32, kind="ExternalInput")
        inp[f"convb_{d}"] = nc.dram_tensor(f"convb_{d}", [DQ, 1], f32, kind="ExternalInput")
        inp[f"wxpT_{d}"] = nc.dram_tensor(f"wxpT_{d}", [DQ, 96], bf16, kind="ExternalInput")
        inp[f"wdtT_{d}"] = nc.dram_tensor(f"wdtT_{d}", [DTR, DQ], bf16, kind="ExternalInput")
        inp[f"dtb_{d}"] = nc.dram_tensor(f"dtb_{d}", [DQ, 1], f32, kind="ExternalInput")
        inp[f"aneg_{d}"] = nc.dram_tensor(f"aneg_{d}", [DQ, NST], f32, kind="ExternalInput")
        inp[f"dv_{d}"] = nc.dram_tensor(f"dv_{d}", [DQ, 1], f32, kind="ExternalInput")
        inp[f"woutT_{d}"] = nc.dram_tensor(f"woutT_{d}", [DQ, DM], bf16, kind="ExternalInput")
    inp["gamma"] = nc.dram_tensor("gamma", [1, DM], f32, kind="ExternalInput")
    inp["beta"] = nc.dram_tensor("beta", [1, DM], f32, kind="ExternalInput")
    out_q = nc.dram_tensor("out_q", [L // 4, DM], f32, kind="ExternalOutput")

    QUADS = [[0, 1, 2, 3], [4, 5, 6, 7]]

    with tile.TileContext(nc) as tc:
        glob = tc.alloc_tile_pool(name="glob", bufs=1)
        dram = tc.alloc_tile_pool(name="dram", bufs=1, space="DRAM")
        pmm = tc.alloc_tile_pool(name="pmm", bufs=3, space="PSUM")
        py = tc.alloc_tile_pool(name="py", bufs=4, space="PSUM")

        # ---- persistent constants / activations ----
        ident = glob.tile([128, 128], bf16)
        make_identity(nc, ident)
        convb_sb, dtb_sb, dv_sb, aneg_sb, diagD = {}, {}, {}, {}, {}
        for d in DIRS:
            convb_sb[d] = glob.tile([128, J], f32, tag=f"convb{d}", name=f"convb{d}")
            nc.sync.dma_start(out=convb_sb[d],
                              in_=inp[f"convb_{d}"].rearrange("(j p) o -> p (j o)", p=128))
            dtb_sb[d] = glob.tile([128, J], f32, tag=f"dtb{d}", name=f"dtb{d}")
            nc.sync.dma_start(out=dtb_sb[d],
                              in_=inp[f"dtb_{d}"].rearrange("(j p) o -> p (j o)", p=128))
            dv_sb[d] = glob.tile([128, J], f32, tag=f"dv{d}", name=f"dv{d}")
            nc.sync.dma_start(out=dv_sb[d],
                              in_=inp[f"dv_{d}"].rearrange("(j p) o -> p (j o)", p=128))
            aneg_sb[d] = glob.tile([128, J, NST], f32, tag=f"aneg{d}", name=f"aneg{d}")
            nc.sync.dma_start(out=aneg_sb[d],
                              in_=inp[f"aneg_{d}"].rearrange("(j p) n -> p j n", p=128))
            diagD[d] = glob.tile([128, J, 128], bf16, tag=f"diagD{d}", name=f"diagD{d}")
            for j in range(J):
                nc.vector.tensor_scalar_mul(out=diagD[d][:, j, :], in0=ident,
                                            scalar1=dv_sb[d][:, j:j + 1])

        xc_sb = {d: glob.tile([128, J, L], bf16, tag=f"xc{d}", name=f"xc{d}") for d in DIRS}
        zs_sb = {d: glob.tile([128, J, L], bf16, tag=f"zs{d}", name=f"zs{d}") for d in DIRS}
        yg_sb = {d: glob.tile([128, J, L], bf16, tag=f"yg{d}", name=f"yg{d}") for d in DIRS}
        dbl_sb = {d: glob.tile([96, L], bf16, tag=f"dbl{d}", name=f"dbl{d}") for d in DIRS}
        oF = glob.tile([128, 8, DM], f16)      # fwd out_proj accumulator [t, dm]
        oFb = glob.tile([128, 8, DM], f16)     # fwd + bwd-jp0 accumulator

        dbl_in = {d: dram.tile([96, L], bf16, tag=f"dbli{d}", name=f"dbli{d}") for d in DIRS}
        dbl_out = {d: dram.tile([96, L], bf16, tag=f"dblo{d}", name=f"dblo{d}") for d in DIRS}
        op_in = dram.tile([L, DM], f16)
        op_out = dram.tile([L // 4, DM], f16)

        # ================= phase A (per dir): in_proj, conv, x_proj =========
        with tc.tile_pool(name="pA", bufs=1) as pA:
            xT_sb = {}
            for d in DIRS:
                xT_sb[d] = pA.tile([128, NK, L], bf16, tag=f"xT{d}", name=f"xT{d}")
                nc.sync.dma_start(out=xT_sb[d],
                                  in_=inp[f"xT_{d}"].rearrange("(k p) t -> p k t", p=128))

            for d in DIRS:
                convw = pA.tile([128, J, DCONV], f32, tag="convw", name="convw")
                nc.sync.dma_start(out=convw,
                                  in_=inp[f"convw_{d}"].rearrange("(j p) k -> p j k", p=128))
                diagW = pA.tile([128, J, DCONV, 128], bf16, tag="diagW", name="diagW")
                for j in range(J):
                    for k in range(DCONV):
                        nc.vector.tensor_scalar_mul(out=diagW[:, j, k, :], in0=ident,
                                                    scalar1=convw[:, j, k:k + 1])
                wxi = pA.tile([128, NK, DQ], bf16, tag="wxi", name="wxi")
                nc.sync.dma_start(out=wxi,
                                  in_=inp[f"wxiT_{d}"].rearrange("(k p) c -> p k c", p=128))
                wxp = pA.tile([128, J, 96], bf16, tag="wxp", name="wxp")
                nc.sync.dma_start(out=wxp,
                                  in_=inp[f"wxpT_{d}"].rearrange("(j p) r -> p j r", p=128))

                xi_sb = {}
                for j in range(J):
                    xi_sb[j] = pA.tile([128, L + 3], bf16, tag=f"xi{j}", name=f"xi{j}")
                    nc.vector.memset(xi_sb[j][:, 0:3], 0.0)
                    for tb in range(2):
                        pxi = pmm.tile([128, 512], f32, tag="mm", name="mm")
                        for k in range(NK):
                            nc.tensor.matmul(out=pxi[:, :],
                                             lhsT=wxi[:, k, ts(j, 128)],
                                             rhs=xT_sb[d][:, k, ts(tb, 512)],
                                             start=(k == 0), stop=(k == NK - 1))
                        nc.scalar.copy(out=xi_sb[j][:, 3 + tb * 512: 3 + (tb + 1) * 512],
                                       in_=pxi[:, :])
                    # causal depthwise conv as diag-matmuls, silu(+bias)
                    for tb in range(2):
                        pxc = pmm.tile([128, 512], f32, tag="mm", name="mm")
                        for k in range(DCONV):
                            nc.tensor.matmul(out=pxc[:, :],
                                             lhsT=diagW[:, j, k, :],
                                             rhs=xi_sb[j][:, tb * 512 + k: tb * 512 + k + 512],
                                             start=(k == 0), stop=(k == DCONV - 1))
                        nc.scalar.activation(out=xc_sb[d][:, j, ts(tb, 512)], in_=pxc[:, :],
                                             func=AF.Silu, bias=convb_sb[d][:, j:j + 1])
                # x_proj partial + immediate per-dir AllReduce
                dblp = pA.tile([96, L], bf16, tag="dblp", name="dblp")
                for tb in range(2):
                    pdb = pmm.tile([96, 512], f32, tag="mm", name="mm")
                    for j in range(J):
                        nc.tensor.matmul(out=pdb[:, :],
                                         lhsT=wxp[:, j, :],
                                         rhs=xc_sb[d][:, j, ts(tb, 512)],
                                         start=(j == 0), stop=(j == J - 1))
                    nc.scalar.copy(out=dblp[:, ts(tb, 512)], in_=pdb[:, :])
                nc.sync.dma_start(out=dbl_in[d][:, :], in_=dblp)
                if no_collectives:
                    nc.sync.dma_start(out=dbl_out[d][:, :], in_=dbl_in[d][:, :])
                else:
                    nc.gpsimd.collective_compute(
                        "AllReduce", ALU.add, replica_groups=QUADS,
                        ins=[dbl_in[d].opt()], outs=[dbl_out[d].opt()])
                nc.sync.dma_start(out=dbl_sb[d], in_=dbl_out[d][:, :])

                # z projection + silu (after x_proj so the AllReduce fires early)
                wz = pA.tile([128, NK, DQ], bf16, tag="wz", name="wz")
                nc.sync.dma_start(out=wz,
                                  in_=inp[f"wzT_{d}"].rearrange("(k p) c -> p k c", p=128))
                for j in range(J):
                    for tb in range(2):
                        pz = pmm.tile([128, 512], f32, tag="mm", name="mm")
                        for k in range(NK):
                            nc.tensor.matmul(out=pz[:, :],
                                             lhsT=wz[:, k, ts(j, 128)],
                                             rhs=xT_sb[d][:, k, ts(tb, 512)],
                                             start=(k == 0), stop=(k == NK - 1))
                        nc.scalar.activation(out=zs_sb[d][:, j, ts(tb, 512)], in_=pz[:, :],
                                             func=AF.Silu)

        # ================= phase B (per dir): dt_proj, scans, gate, out_proj
        wout = {}
        for d in DIRS:
            wout[d] = glob.tile([128, J, DM], bf16, tag=f"wout{d}", name=f"wout{d}")
            nc.sync.dma_start(out=wout[d],
                              in_=inp[f"woutT_{d}"].rearrange("(j p) m -> p j m", p=128))

        with tc.tile_pool(name="pS", bufs=1) as pS:
            for d in DIRS:
                wdt = pS.tile([DTR, DQ], bf16, tag="wdt", name="wdt")
                nc.sync.dma_start(out=wdt, in_=inp[f"wdtT_{d}"][:, :])
                dtl = dbl_sb[d]

                dt_sb, dtxc_sb = {}, {}
                ygflip = pS.tile([128, J, L], bf16, tag="ygflip", name="ygflip")
                for jp in range(2):          # j-pair rounds (PSUM pressure)
                    js = (2 * jp, 2 * jp + 1)
                    for j in js:
                        dt_sb[j] = pS.tile([128, L], f32, tag=f"dt{j}", name=f"dt{j}")
                        for tb in range(2):
                            pdt = pmm.tile([128, 512], f32, tag="mm", name="mm")
                            nc.tensor.matmul(out=pdt[:, :],
                                             lhsT=wdt[:, ts(j, 128)],
                                             rhs=dtl[0:DTR, ts(tb, 512)],
                                             start=True, stop=True)
                            # softplus(x) = ln(exp(x) + 1)
                            dte = pS.tile([128, 512], f32, tag="dte", name="dte",
                                          bufs=2)
                            nc.scalar.activation(out=dte, in_=pdt[:, :], func=AF.Exp,
                                                 bias=dtb_sb[d][:, j:j + 1])
                            nc.scalar.activation(out=dt_sb[j][:, ts(tb, 512)],
                                                 in_=dte, func=AF.Ln, bias=1.0)
                        dtxc_sb[j] = pS.tile([128, L], bf16, tag=f"dtxc{j}",
                                             name=f"dtxc{j}")
                        nc.vector.tensor_tensor(out=dtxc_sb[j],
                                                in0=dt_sb[j],
                                                in1=xc_sb[d][:, j, :], op=ALU.mult)

                    pyt = {}
                    for j in js:
                        for tb in range(2):
                            pyt[(j, tb)] = py.tile([128, 512], f32, tag="y", name="y")

                    for n in range(NST):
                        brep = pS.tile([128, L], bf16, tag="brep", name="brep", bufs=3)
                        nc.sync.dma_start(
                            out=brep,
                            in_=bass.AP(tensor=dbl_out[d].tensor,
                                        offset=dbl_out[d].opt().offset + (DTR + n) * L,
                                        ap=[[0, 128], [1, L]]))
                        crep = pS.tile([128, L], bf16, tag="crep", name="crep", bufs=3)
                        nc.sync.dma_start(
                            out=crep,
                            in_=bass.AP(tensor=dbl_out[d].tensor,
                                        offset=dbl_out[d].opt().offset + (DTR + NST + n) * L,
                                        ap=[[0, 128], [1, L]]))
                        for j in js:
                            dA = pS.tile([128, L], f32, tag="dA", name="dA", bufs=3)
                            nc.scalar.activation(out=dA, in_=dt_sb[j], func=AF.Exp,
                                                 scale=aneg_sb[d][:, j, n:n + 1])
                            dBu = pS.tile([128, L], bf16, tag="dBu", name="dBu", bufs=3)
                            nc.vector.tensor_tensor(out=dBu, in0=dtxc_sb[j],
                                                    in1=brep, op=ALU.mult)
                            h = pS.tile([128, L], bf16, tag="h", name="h", bufs=3)
                            nc.vector.tensor_tensor_scan(out=h, data0=dA, data1=dBu,
                                                         initial=0.0, op0=ALU.mult,
                                                         op1=ALU.add)
                            hC = pS.tile([128, L], bf16, tag="hC", name="hC", bufs=3)
                            nc.vector.tensor_tensor(out=hC, in0=h, in1=crep, op=ALU.mult)
                            for tb in range(2):
                                nc.tensor.matmul(out=pyt[(j, tb)][:, :], lhsT=ident,
                                                 rhs=hC[:, ts(tb, 512)],
                                                 start=(n == 0), stop=False)
                    # skip term D*xc, gate with silu(z)
                    for j in js:
                        for tb in range(2):
                            nc.tensor.matmul(out=pyt[(j, tb)][:, :],
                                             lhsT=diagD[d][:, j, :],
                                             rhs=xc_sb[d][:, j, ts(tb, 512)],
                                             start=False, stop=True)
                            nc.vector.tensor_tensor(out=yg_sb[d][:, j, ts(tb, 512)],
                                                    in0=pyt[(j, tb)][:, :],
                                                    in1=zs_sb[d][:, j, ts(tb, 512)],
                                                    op=ALU.mult)
                    if d == "b":
                        # flip this round's gated output, partial out_proj now:
                        # jp0 accumulates into oF, jp1 finishes and ships fp16
                        for j in js:
                            nc.vector.tensor_copy(out=ygflip[:, j, :],
                                                  in_=yg_sb["b"][:, j, ::-1])
                        for ti in range(8):
                            for mh in range(2):
                                po = pmm.tile([128, 512], f32, tag="mm", name="mm")
                                for kk, j in enumerate(js):
                                    nc.tensor.matmul(out=po[:, :],
                                                     lhsT=ygflip[:, j, ts(ti, 128)],
                                                     rhs=wout["b"][:, j, ts(mh, 512)],
                                                     start=(kk == 0), stop=(kk == 1))
                                if jp == 0:
                                    nc.vector.tensor_tensor(
                                        out=oFb[:, ti, ts(mh, 512)], in0=po[:, :],
                                        in1=oF[:, ti, ts(mh, 512)], op=ALU.add)
                                else:
                                    osb = pS.tile([128, 512], f16, tag="osb",
                                                  name="osb", bufs=3)
                                    nc.vector.tensor_tensor(
                                        out=osb, in0=po[:, :],
                                        in1=oFb[:, ti, ts(mh, 512)], op=ALU.add)
                                    nc.sync.dma_start(
                                        out=op_in[ts(ti, 128), ts(mh, 512)], in_=osb)

                # ---- fwd out_proj into the SBUF accumulator (overlaps bwd) ----
                if d == "f":
                    for ti in range(8):
                        for mh in range(2):
                            po = pmm.tile([128, 512], f32, tag="mm", name="mm")
                            for j in range(J):
                                nc.tensor.matmul(out=po[:, :],
                                                 lhsT=yg_sb["f"][:, j, ts(ti, 128)],
                                                 rhs=wout["f"][:, j, ts(mh, 512)],
                                                 start=(j == 0), stop=(j == J - 1))
                            nc.scalar.copy(out=oF[:, ti, ts(mh, 512)], in_=po[:, :])

        # ================= ReduceScatter + LayerNorm ========================
        with tc.tile_pool(name="pO", bufs=1) as pO:
            if no_collectives:
                nc.sync.dma_start(out=op_out[:, :], in_=op_in[0:L // 4, :])
            else:
                nc.gpsimd.collective_compute("ReduceScatter", ALU.add,
                                             replica_groups=QUADS,
                                             ins=[op_in.opt()], outs=[op_out.opt()])

            epst = pO.tile([128, 1], f32, tag="epst", name="epst")
            nc.vector.memset(epst, EPS)
            grep = pO.tile([128, DM], f32, tag="grep", name="grep")
            nc.sync.dma_start(out=grep, in_=bass.AP(tensor=inp["gamma"][:, :].tensor,
                                                    offset=0, ap=[[0, 128], [1, DM]]))
            brep2 = pO.tile([128, DM], f32, tag="brep2", name="brep2")
            nc.sync.dma_start(out=brep2, in_=bass.AP(tensor=inp["beta"][:, :].tensor,
                                                     offset=0, ap=[[0, 128], [1, DM]]))
            for half in range(2):
                xln = pO.tile([128, DM], f32, tag="xln", name="xln", bufs=2)
                nc.gpsimd.dma_start(out=xln, in_=op_out[ts(half, 128), :])
                stats = pO.tile([128, 2, 6], f32, tag="stats", name="stats", bufs=2)
                for sg in range(2):
                    nc.vector.bn_stats(out=stats[:, sg, :], in_=xln[:, ts(sg, 512)])
                mv = pO.tile([128, 2], f32, tag="mv", name="mv", bufs=2)
                nc.vector.bn_aggr(out=mv, in_=stats)
                rstd = pO.tile([128, 1], f32, tag="rstd", name="rstd", bufs=2)
                nc.scalar.activation(out=rstd, in_=mv[:, 1:2], func=AF.Sqrt,
                                     bias=epst[:, 0:1])
                nc.vector.reciprocal(out=rstd, in_=rstd)
                xn = pO.tile([128, DM], f32, tag="xn", name="xn", bufs=2)
                nc.vector.tensor_scalar(out=xn, in0=xln, scalar1=mv[:, 0:1],
                                        scalar2=rstd, op0=ALU.subtract, op1=ALU.mult)
                nc.vector.tensor_tensor(out=xn, in0=xn, in1=grep, op=ALU.mult)
                nc.vector.tensor_tensor(out=xn, in0=xn, in1=brep2, op=ALU.add)
                nc.sync.dma_start(out=out_q[ts(half, 128), :], in_=xn)

        py.release()
        pmm.release()
        dram.release()
        glob.release()

    nc.compile()
    return nc


def _prep_in_maps(inputs):
    x = np.asarray(inputs["x"], np.float32)
    in_maps = []
    for b in range(B_):
        xTf = np.ascontiguousarray(x[b].T).astype(ml_dtypes.bfloat16)
        xTb = np.ascontiguousarray(x[b][::-1, :].T).astype(ml_dtypes.bfloat16)
        for q in range(4):
            sl = slice(q * DQ, (q + 1) * DQ)
            m = {"xT_f": xTf, "xT_b": xTb,
                 "gamma": np.asarray(inputs["ln_gamma"], np.float32).reshape(1, DM),
                 "beta": np.asarray(inputs["ln_beta"], np.float32).reshape(1, DM)}
            for d, pref in (("f", "fwd_"), ("b", "bwd_")):
                W = {k[len(pref):]: np.asarray(v, np.float32)
                     for k, v in inputs.items() if k.startswith(pref)}
                m[f"wxiT_{d}"] = np.ascontiguousarray(W["in_proj_w"][sl, :].T).astype(ml_dtypes.bfloat16)
                m[f"wzT_{d}"] = np.ascontiguousarray(W["in_proj_w"][DI + q * DQ: DI + (q + 1) * DQ, :].T).astype(ml_dtypes.bfloat16)
                m[f"convw_{d}"] = np.ascontiguousarray(W["conv_w"][sl])
                m[f"convb_{d}"] = np.ascontiguousarray(W["conv_b"][sl]).reshape(DQ, 1)
                m[f"wxpT_{d}"] = np.ascontiguousarray(W["x_proj_w"][:, sl].T).astype(ml_dtypes.bfloat16)
                m[f"wdtT_{d}"] = np.ascontiguousarray(W["dt_proj_w"][sl, :].T).astype(ml_dtypes.bfloat16)
                m[f"dtb_{d}"] = np.ascontiguousarray(W["dt_proj_b"][sl]).reshape(DQ, 1)
                m[f"aneg_{d}"] = np.ascontiguousarray(-np.exp(W["A_log"][sl]))
                m[f"dv_{d}"] = np.ascontiguousarray(W["D"][sl]).reshape(DQ, 1)
                m[f"woutT_{d}"] = np.ascontiguousarray(W["out_proj_w"][:, sl].T).astype(ml_dtypes.bfloat16)
            in_maps.append(m)
    return in_maps


def kernel(**inputs):
    if "nc" not in _cached:
        _cached["nc"] = build()
    nc = _cached["nc"]
    in_maps = _prep_in_maps(inputs)
    res = run_bass_kernel_spmd(nc, in_maps, core_ids=list(range(8)),
                               **_cached.get("run_kwargs", {}))
    _cached["last_result"] = res
    out = np.empty((B_, L, DM), np.float32)
    for b in range(B_):
        for q in range(4):
            out[b, q * 256:(q + 1) * 256, :] = res.results[4 * b + q]["out_q"]
    return out


# revision 14
# speedup vs baseline: 1.2774x; 1.0023x over previous
"""BiMamba block Trainium2 kernel (8 NeuronCores, SPMD).  v2

Problem: x (2, 1024, 1024) -> bidirectional Mamba (fwd + bwd over flipped
sequence, independent weights) summed, then LayerNorm over d_model.

Sharding: core = (batch b, d_inner quarter q).  Each core processes BOTH
directions for its 512 of 2048 inner channels over the full sequence
(L = 1024), so the selective scan never crosses cores.

v2 structure (vs v1): per-direction AllReduce so fwd scans start while bwd
in_proj still runs on PE (keeps PE dense -> HAM stays warm); dBu multiply on
GPSIMD and 1/4 of the h*C multiplies on GPSIMD to relieve DVE (the
tensor_tensor_scan runs at ~2.2 cyc/el and dominates DVE); out_proj(fwd)
runs under the bwd scans into an SBUF accumulator; ReduceScatter payload is
fp16; dt_proj in bf16.
"""

import sys

sys.path.insert(0, "/opt/trn_rl_repo")

import numpy as np
import ml_dtypes

import concourse.bass as bass
import concourse.tile as tile
import concourse.mybir as mybir
from concourse import bacc
from concourse.bass import ts
from concourse.bass_utils import run_bass_kernel_spmd
from concourse.masks import make_identity

AF = mybir.ActivationFunctionType
ALU = mybir.AluOpType
bf16 = mybir.dt.bfloat16
f16 = mybir.dt.float16
f32 = mybir.dt.float32

B_, L, DM = 2, 1024, 1024
DI, NST, DCONV, DTR = 2048, 16, 4, 64
DQ = 512          # d_inner channels per core
J = DQ // 128     # 4 partition tiles per core
NK = DM // 128    # 8 contraction tiles for d_model
EPS = 1e-5
DIRS = ("f", "b")

_cached = {}


def build(no_collectives=False):
    nc = bacc.Bacc("TRN2", target_bir_lowering=False, debug=False, num_devices=8)

    inp = {}
    for d in DIRS:
        inp[f"xT_{d}"] = nc.dram_tensor(f"xT_{d}", [DM, L], bf16, kind="ExternalInput")
        inp[f"wxiT_{d}"] = nc.dram_tensor(f"wxiT_{d}", [DM, DQ], bf16, kind="ExternalInput")
        inp[f"wzT_{d}"] = nc.dram_tensor(f"wzT_{d}", [DM, DQ], bf16, kind="ExternalInput")
        inp[f"convw_{d}"] = nc.dram_tensor(f"convw_{d}", [DQ, DCONV], f32, kind="ExternalInput")
        inp[f"convb_{d}"] = nc.dram_tensor(f"convb_{d}", [DQ, 1], f32, kind="ExternalInput")
        inp[f"wxpT_{d}"] = nc.dram_tensor(f"wxpT_{d}", [DQ, 96], bf16, kind="ExternalInput")
        inp[f"wdtT_{d}"] = nc.dram_tensor(f"wdtT_{d}", [DTR, DQ], bf16, kind="ExternalInput")
        inp[f"dtb_{d}"] = nc.dram_tensor(f"dtb_{d}", [DQ, 1], f32, kind="ExternalInput")
        inp[f"aneg_{d}"] = nc.dram_tensor(f"aneg_{d}", [DQ, NST], f32, kind="ExternalInput")
        inp[f"dv_{d}"] = nc.dram_tensor(f"dv_{d}", [DQ, 1], f32, kind="ExternalInput")
        inp[f"woutT_{d}"] = nc.dram_tensor(f"woutT_{d}", [DQ, DM], bf16, kind="ExternalInput")
    inp["gamma"] = nc.dram_tensor("gamma", [1, DM], f32, kind="ExternalInput")
    inp["beta"] = nc.dram_tensor("beta", [1, DM], f32, kind="ExternalInput")
    out_q = nc.dram_tensor("out_q", [L // 4, DM], f32, kind="ExternalOutput")

    QUADS = [[0, 1, 2, 3], [4, 5, 6, 7]]

    with tile.TileContext(nc) as tc:
        glob = tc.alloc_tile_pool(name="glob", bufs=1)
        dram = tc.alloc_tile_pool(name="dram", bufs=1, space="DRAM")
        pmm = tc.alloc_tile_pool(name="pmm", bufs=3, space="PSUM")
        py = tc.alloc_tile_pool(name="py", bufs=4, space="PSUM")

        # ---- persistent constants / activations ----
        ident = glob.tile([128, 128], bf16)
        make_identity(nc, ident)
        convb_sb, dtb_sb, dv_sb, aneg_sb, diagD = {}, {}, {}, {}, {}
        for d in DIRS:
            convb_sb[d] = glob.tile([128, J], f32, tag=f"convb{d}", name=f"convb{d}")
            nc.sync.dma_start(out=convb_sb[d],
                              in_=inp[f"convb_{d}"].rearrange("(j p) o -> p (j o)", p=128))
            dtb_sb[d] = glob.tile([128, J], f32, tag=f"dtb{d}", name=f"dtb{d}")
            nc.sync.dma_start(out=dtb_sb[d],
                              in_=inp[f"dtb_{d}"].rearrange("(j p) o -> p (j o)", p=128))
            dv_sb[d] = glob.tile([128, J], f32, tag=f"dv{d}", name=f"dv{d}")
            nc.sync.dma_start(out=dv_sb[d],
                              in_=inp[f"dv_{d}"].rearrange("(j p) o -> p (j o)", p=128))
            aneg_sb[d] = glob.tile([128, J, NST], f32, tag=f"aneg{d}", name=f"aneg{d}")
            nc.sync.dma_start(out=aneg_sb[d],
                              in_=inp[f"aneg_{d}"].rearrange("(j p) n -> p j n", p=128))
            diagD[d] = glob.tile([128, J, 128], bf16, tag=f"diagD{d}", name=f"diagD{d}")
            for j in range(J):
                nc.vector.tensor_scalar_mul(out=diagD[d][:, j, :], in0=ident,
                                            scalar1=dv_sb[d][:, j:j + 1])

        xc_sb = {d: glob.tile([128, J, L], bf16, tag=f"xc{d}", name=f"xc{d}") for d in DIRS}
        zs_sb = {d: glob.tile([128, J, L], bf16, tag=f"zs{d}", name=f"zs{d}") for d in DIRS}
        yg_sb = {d: glob.tile([128, J, L], bf16, tag=f"yg{d}", name=f"yg{d}") for d in DIRS}
        dbl_sb = {d: glob.tile([96, L], bf16, tag=f"dbl{d}", name=f"dbl{d}") for d in DIRS}
        oF = glob.tile([128, 8, DM], f16)      # fwd out_proj accumulator [t, dm]
        oFb = glob.tile([128, 8, DM], f16)     # fwd + bwd-jp0 accumulator

        dbl_in = {d: dram.tile([96, L], bf16, tag=f"dbli{d}", name=f"dbli{d}") for d in DIRS}
        dbl_out = {d: dram.tile([96, L], bf16, tag=f"dblo{d}", name=f"dblo{d}") for d in DIRS}
        op_in = dram.tile([2, L, 512], f16)
        op_out = dram.tile([2, L // 4, 512], f16)

        # ================= phase A (per dir): in_proj, conv, x_proj =========
        with tc.tile_pool(name="pA", bufs=1) as pA:
            xT_sb = {}
            for d in DIRS:
                xT_sb[d] = pA.tile([128, NK, L], bf16, tag=f"xT{d}", name=f"xT{d}")
                nc.sync.dma_start(out=xT_sb[d],
                                  in_=inp[f"xT_{d}"].rearrange("(k p) t -> p k t", p=128))

            for d in DIRS:
                convw = pA.tile([128, J, DCONV], f32, tag="convw", name="convw")
                nc.sync.dma_start(out=convw,
                                  in_=inp[f"convw_{d}"].rearrange("(j p) k -> p j k", p=128))
                diagW = pA.tile([128, J, DCONV, 128], bf16, tag="diagW", name="diagW")
                for j in range(J):
                    for k in range(DCONV):
                        nc.vector.tensor_scalar_mul(out=diagW[:, j, k, :], in0=ident,
                                                    scalar1=convw[:, j, k:k + 1])
                wxi = pA.tile([128, NK, DQ], bf16, tag="wxi", name="wxi")
                nc.sync.dma_start(out=wxi,
                                  in_=inp[f"wxiT_{d}"].rearrange("(k p) c -> p k c", p=128))
                wxp = pA.tile([128, J, 96], bf16, tag="wxp", name="wxp")
                nc.sync.dma_start(out=wxp,
                                  in_=inp[f"wxpT_{d}"].rearrange("(j p) r -> p j r", p=128))

                xi_sb = {}
                for j in range(J):
                    xi_sb[j] = pA.tile([128, L + 3], bf16, tag=f"xi{j}", name=f"xi{j}")
                    nc.vector.memset(xi_sb[j][:, 0:3], 0.0)
                    for tb in range(2):
                        pxi = pmm.tile([128, 512], f32, tag="mm", name="mm")
                        for k in range(NK):
                            nc.tensor.matmul(out=pxi[:, :],
                                             lhsT=wxi[:, k, ts(j, 128)],
                                             rhs=xT_sb[d][:, k, ts(tb, 512)],
                                             start=(k == 0), stop=(k == NK - 1))
                        nc.scalar.copy(out=xi_sb[j][:, 3 + tb * 512: 3 + (tb + 1) * 512],
                                       in_=pxi[:, :])
                    # causal depthwise conv as diag-matmuls, silu(+bias)
                    for tb in range(2):
                        pxc = pmm.tile([128, 512], f32, tag="mm", name="mm")
                        for k in range(DCONV):
                            nc.tensor.matmul(out=pxc[:, :],
                                             lhsT=diagW[:, j, k, :],
                                             rhs=xi_sb[j][:, tb * 512 + k: tb * 512 + k + 512],
                                             start=(k == 0), stop=(k == DCONV - 1))
                        nc.scalar.activation(out=xc_sb[d][:, j, ts(tb, 512)], in_=pxc[:, :],
                                             func=AF.Silu, bias=convb_sb[d][:, j:j + 1])
                # x_proj partial + immediate per-dir AllReduce
                dblp = pA.tile([96, L], bf16, tag="dblp", name="dblp")
                for tb in range(2):
                    pdb = pmm.tile([96, 512], f32, tag="mm", name="mm")
                    for j in range(J):
                        nc.tensor.matmul(out=pdb[:, :],
                                         lhsT=wxp[:, j, :],
                                         rhs=xc_sb[d][:, j, ts(tb, 512)],
                                         start=(j == 0), stop=(j == J - 1))
                    nc.scalar.copy(out=dblp[:, ts(tb, 512)], in_=pdb[:, :])
                nc.sync.dma_start(out=dbl_in[d][:, :], in_=dblp)
                if no_collectives:
                    nc.sync.dma_start(out=dbl_out[d][:, :], in_=dbl_in[d][:, :])
                else:
                    nc.gpsimd.collective_compute(
                        "AllReduce", ALU.add, replica_groups=QUADS,
                        ins=[dbl_in[d].opt()], outs=[dbl_out[d].opt()])
                nc.sync.dma_start(out=dbl_sb[d], in_=dbl_out[d][:, :])

                # z projection + silu (after x_proj so the AllReduce fires early)
                wz = pA.tile([128, NK, DQ], bf16, tag="wz", name="wz")
                nc.sync.dma_start(out=wz,
                                  in_=inp[f"wzT_{d}"].rearrange("(k p) c -> p k c", p=128))
                for j in range(J):
                    for tb in range(2):
                        pz = pmm.tile([128, 512], f32, tag="mm", name="mm")
                        for k in range(NK):
                            nc.tensor.matmul(out=pz[:, :],
                                             lhsT=wz[:, k, ts(j, 128)],
                                             rhs=xT_sb[d][:, k, ts(tb, 512)],
                                             start=(k == 0), stop=(k == NK - 1))
                        nc.scalar.activation(out=zs_sb[d][:, j, ts(tb, 512)], in_=pz[:, :],
                                             func=AF.Silu)

        # ================= phase B (per dir): dt_proj, scans, gate, out_proj
        wout = {}
        for d in DIRS:
            wout[d] = glob.tile([128, J, DM], bf16, tag=f"wout{d}", name=f"wout{d}")
            nc.sync.dma_start(out=wout[d],
                              in_=inp[f"woutT_{d}"].rearrange("(j p) m -> p j m", p=128))

        with tc.tile_pool(name="pS", bufs=1) as pS:
            for d in DIRS:
                wdt = pS.tile([DTR, DQ], bf16, tag="wdt", name="wdt")
                nc.sync.dma_start(out=wdt, in_=inp[f"wdtT_{d}"][:, :])
                dtl = dbl_sb[d]

                dt_sb, dtxc_sb = {}, {}
                ygflip = pS.tile([128, J, L], bf16, tag="ygflip", name="ygflip")
                for jp in range(2):          # j-pair rounds (PSUM pressure)
                    js = (2 * jp, 2 * jp + 1)
                    for j in js:
                        dt_sb[j] = pS.tile([128, L], f32, tag=f"dt{j}", name=f"dt{j}")
                        for tb in range(2):
                            pdt = pmm.tile([128, 512], f32, tag="mm", name="mm")
                            nc.tensor.matmul(out=pdt[:, :],
                                             lhsT=wdt[:, ts(j, 128)],
                                             rhs=dtl[0:DTR, ts(tb, 512)],
                                             start=True, stop=True)
                            # softplus(x) = ln(exp(x) + 1)
                            dte = pS.tile([128, 512], f32, tag="dte", name="dte",
                                          bufs=2)
                            nc.scalar.activation(out=dte, in_=pdt[:, :], func=AF.Exp,
                                                 bias=dtb_sb[d][:, j:j + 1])
                            nc.scalar.activation(out=dt_sb[j][:, ts(tb, 512)],
                                                 in_=dte, func=AF.Ln, bias=1.0)
                        dtxc_sb[j] = None  # stored in dtxc_pair below

                    dtxc_pair = pS.tile([128, 2, L], bf16, tag="dtxcp",
                                        name="dtxcp", bufs=2)
                    for jj, j in enumerate(js):
                        nc.vector.tensor_tensor(out=dtxc_pair[:, jj, :],
                                                in0=dt_sb[j],
                                                in1=xc_sb[d][:, j, :], op=ALU.mult)
                    pyt = {}
                    for j in js:
                        for tb in range(2):
                            pyt[(j, tb)] = py.tile([128, 512], f32, tag="y", name="y")

                    for n in range(NST):
                        brep = pS.tile([128, L], bf16, tag="brep", name="brep", bufs=3)
                        nc.sync.dma_start(
                            out=brep,
                            in_=bass.AP(tensor=dbl_out[d].tensor,
                                        offset=dbl_out[d].opt().offset + (DTR + n) * L,
                                        ap=[[0, 128], [1, L]]))
                        crep = pS.tile([128, L], bf16, tag="crep", name="crep", bufs=3)
                        nc.sync.dma_start(
                            out=crep,
                            in_=bass.AP(tensor=dbl_out[d].tensor,
                                        offset=dbl_out[d].opt().offset + (DTR + NST + n) * L,
                                        ap=[[0, 128], [1, L]]))
                        brep2d = bass.AP(tensor=brep.tensor, offset=brep.offset,
                                         ap=[brep.ap[0], [0, 2], [1, L]])
                        crep2d = bass.AP(tensor=crep.tensor, offset=crep.offset,
                                         ap=[crep.ap[0], [0, 2], [1, L]])
                        # fused across the j-pair: one wide TT for dBu and h*C
                        dBu = pS.tile([128, 2, L], bf16, tag="dBu", name="dBu", bufs=3)
                        nc.vector.tensor_tensor(out=dBu, in0=dtxc_pair,
                                                in1=brep2d, op=ALU.mult)
                        dA = pS.tile([128, 2, L], f32, tag="dA", name="dA", bufs=2)
                        h = pS.tile([128, 2, L], bf16, tag="h", name="h", bufs=3)
                        for jj, j in enumerate(js):
                            nc.scalar.activation(out=dA[:, jj, :], in_=dt_sb[j],
                                                 func=AF.Exp,
                                                 scale=aneg_sb[d][:, j, n:n + 1])
                            nc.vector.tensor_tensor_scan(out=h[:, jj, :],
                                                         data0=dA[:, jj, :],
                                                         data1=dBu[:, jj, :],
                                                         initial=0.0, op0=ALU.mult,
                                                         op1=ALU.add)
                        hC = pS.tile([128, 2, L], bf16, tag="hC", name="hC", bufs=3)
                        nc.vector.tensor_tensor(out=hC, in0=h, in1=crep2d, op=ALU.mult)
                        for jj, j in enumerate(js):
                            for tb in range(2):
                                nc.tensor.matmul(out=pyt[(j, tb)][:, :], lhsT=ident,
                                                 rhs=hC[:, jj, ts(tb, 512)],
                                                 start=(n == 0), stop=False)
                    # skip term D*xc, gate with silu(z)
                    for j in js:
                        for tb in range(2):
                            nc.tensor.matmul(out=pyt[(j, tb)][:, :],
                                             lhsT=diagD[d][:, j, :],
                                             rhs=xc_sb[d][:, j, ts(tb, 512)],
                                             start=False, stop=True)
                            nc.vector.tensor_tensor(out=yg_sb[d][:, j, ts(tb, 512)],
                                                    in0=pyt[(j, tb)][:, :],
                                                    in1=zs_sb[d][:, j, ts(tb, 512)],
                                                    op=ALU.mult)
                    if d == "b":
                        # flip this round's gated output, partial out_proj now:
                        # jp0 accumulates into oF, jp1 finishes and ships fp16
                        # (mh-major so the first ReduceScatter half fires early)
                        for j in js:
                            nc.vector.tensor_copy(out=ygflip[:, j, :],
                                                  in_=yg_sb["b"][:, j, ::-1])
                        for mh in range(2):
                            for ti in range(8):
                                po = pmm.tile([128, 512], f32, tag="mm", name="mm")
                                for kk, j in enumerate(js):
                                    nc.tensor.matmul(out=po[:, :],
                                                     lhsT=ygflip[:, j, ts(ti, 128)],
                                                     rhs=wout["b"][:, j, ts(mh, 512)],
                                                     start=(kk == 0), stop=(kk == 1))
                                if jp == 0:
                                    nc.vector.tensor_tensor(
                                        out=oFb[:, ti, ts(mh, 512)], in0=po[:, :],
                                        in1=oF[:, ti, ts(mh, 512)], op=ALU.add)
                                else:
                                    osb = pS.tile([128, 512], f16, tag="osb",
                                                  name="osb", bufs=3)
                                    nc.vector.tensor_tensor(
                                        out=osb, in0=po[:, :],
                                        in1=oFb[:, ti, ts(mh, 512)], op=ALU.add)
                                    nc.sync.dma_start(
                                        out=op_in[mh, ts(ti, 128), :], in_=osb)
                            if jp == 1:
                                if no_collectives:
                                    nc.sync.dma_start(out=op_out[mh, :, :],
                                                      in_=op_in[mh, 0:L // 4, :])
                                else:
                                    nc.gpsimd.collective_compute(
                                        "ReduceScatter", ALU.add,
                                        replica_groups=QUADS,
                                        ins=[op_in[mh].opt()],
                                        outs=[op_out[mh].opt()])

                # ---- fwd out_proj into the SBUF accumulator (overlaps bwd) ----
                if d == "f":
                    for ti in range(8):
                        for mh in range(2):
                            po = pmm.tile([128, 512], f32, tag="mm", name="mm")
                            for j in range(J):
                                nc.tensor.matmul(out=po[:, :],
                                                 lhsT=yg_sb["f"][:, j, ts(ti, 128)],
                                                 rhs=wout["f"][:, j, ts(mh, 512)],
                                                 start=(j == 0), stop=(j == J - 1))
                            nc.scalar.copy(out=oF[:, ti, ts(mh, 512)], in_=po[:, :])

        # ================= ReduceScatter + LayerNorm ========================
        with tc.tile_pool(name="pO", bufs=1) as pO:
            epst = pO.tile([128, 1], f32, tag="epst", name="epst")
            nc.vector.memset(epst, EPS)
            grep = pO.tile([128, DM], f32, tag="grep", name="grep")
            nc.sync.dma_start(out=grep, in_=bass.AP(tensor=inp["gamma"][:, :].tensor,
                                                    offset=0, ap=[[0, 128], [1, DM]]))
            brep2 = pO.tile([128, DM], f32, tag="brep2", name="brep2")
            nc.sync.dma_start(out=brep2, in_=bass.AP(tensor=inp["beta"][:, :].tensor,
                                                     offset=0, ap=[[0, 128], [1, DM]]))
            for half in range(2):
                xln = pO.tile([128, DM], f32, tag="xln", name="xln", bufs=2)
                for mh in range(2):
                    nc.gpsimd.dma_start(out=xln[:, ts(mh, 512)],
                                        in_=op_out[mh, ts(half, 128), :])
                stats = pO.tile([128, 2, 6], f32, tag="stats", name="stats", bufs=2)
                for sg in range(2):
                    nc.vector.bn_stats(out=stats[:, sg, :], in_=xln[:, ts(sg, 512)])
                mv = pO.tile([128, 2], f32, tag="mv", name="mv", bufs=2)
                nc.vector.bn_aggr(out=mv, in_=stats)
                rstd = pO.tile([128, 1], f32, tag="rstd", name="rstd", bufs=2)
                nc.scalar.activation(out=rstd, in_=mv[:, 1:2], func=AF.Sqrt,
                                     bias=epst[:, 0:1])
                nc.vector.reciprocal(out=rstd, in_=rstd)
                xn = pO.tile([128, DM], f32, tag="xn", name="xn", bufs=2)
                nc.vector.tensor_scalar(out=xn, in0=xln, scalar1=mv[:, 0:1],
                                        scalar2=rstd, op0=ALU.subtract, op1=ALU.mult)
                nc.vector.tensor_tensor(out=xn, in0=xn, in1=grep, op=ALU.mult)
                nc.vector.tensor_tensor(out=xn, in0=xn, in1=brep2, op=ALU.add)
                nc.sync.dma_start(out=out_q[ts(half, 128), :], in_=xn)

        py.release()
        pmm.release()
        dram.release()
        glob.release()

    nc.compile()
    return nc


def _prep_in_maps(inputs):
    x = np.asarray(inputs["x"], np.float32)
    in_maps = []
    for b in range(B_):
        xTf = np.ascontiguousarray(x[b].T).astype(ml_dtypes.bfloat16)
        xTb = np.ascontiguousarray(x[b][::-1, :].T).astype(ml_dtypes.bfloat16)
        for q in range(4):
            sl = slice(q * DQ, (q + 1) * DQ)
            m = {"xT_f": xTf, "xT_b": xTb,
                 "gamma": np.asarray(inputs["ln_gamma"], np.float32).reshape(1, DM),
                 "beta": np.asarray(inputs["ln_beta"], np.float32).reshape(1, DM)}
            for d, pref in (("f", "fwd_"), ("b", "bwd_")):
                W = {k[len(pref):]: np.asarray(v, np.float32)
                     for k, v in inputs.items() if k.startswith(pref)}
                m[f"wxiT_{d}"] = np.ascontiguousarray(W["in_proj_w"][sl, :].T).astype(ml_dtypes.bfloat16)
                m[f"wzT_{d}"] = np.ascontiguousarray(W["in_proj_w"][DI + q * DQ: DI + (q + 1) * DQ, :].T).astype(ml_dtypes.bfloat16)
                m[f"convw_{d}"] = np.ascontiguousarray(W["conv_w"][sl])
                m[f"convb_{d}"] = np.ascontiguousarray(W["conv_b"][sl]).reshape(DQ, 1)
                m[f"wxpT_{d}"] = np.ascontiguousarray(W["x_proj_w"][:, sl].T).astype(ml_dtypes.bfloat16)
                m[f"wdtT_{d}"] = np.ascontiguousarray(W["dt_proj_w"][sl, :].T).astype(ml_dtypes.bfloat16)
                m[f"dtb_{d}"] = np.ascontiguousarray(W["dt_proj_b"][sl]).reshape(DQ, 1)
                m[f"aneg_{d}"] = np.ascontiguousarray(-np.exp(W["A_log"][sl]))
                m[f"dv_{d}"] = np.ascontiguousarray(W["D"][sl]).reshape(DQ, 1)
                m[f"woutT_{d}"] = np.ascontiguousarray(W["out_proj_w"][:, sl].T).astype(ml_dtypes.bfloat16)
            in_maps.append(m)
    return in_maps


def kernel(**inputs):
    if "nc" not in _cached:
        _cached["nc"] = build()
    nc = _cached["nc"]
    in_maps = _prep_in_maps(inputs)
    res = run_bass_kernel_spmd(nc, in_maps, core_ids=list(range(8)),
                               **_cached.get("run_kwargs", {}))
    _cached["last_result"] = res
    out = np.empty((B_, L, DM), np.float32)
    for b in range(B_):
        for q in range(4):
            out[b, q * 256:(q + 1) * 256, :] = res.results[4 * b + q]["out_q"]
    return out
